# revision 1
# baseline (speedup 1.0000x reference)
"""DyBEM layer (histogram binning + embedding sum + linear) on 8 trn2 cores.

Math reduction (same as before)
-------------------------------
ref: xmin/xmax per column n over the batch; u = (x-xmin)/(xmax-xmin+eps);
     bins = cumsum(softmax(bin_logits)); idx = clip(searchsorted(bins, u), 0, 9)
     out  = einsum('bne,fe->bf', embed[idx], W) + IN_DIM * b

Let EW = embed @ W.T (shape [10,64]).  With g[b,k] = #{n : u[b,n] > bins[k]}
(k = 0..8):

  out[b] = IN_DIM*(EW[0] + b) + sum_k g[b,k] * (EW[k+1] - EW[k])

and u[b,n] > bins[k]  <=>  x[b,n] > T[k,n] := xmin[n] + bins[k]*(range[n]+eps).

Latency-oriented schedule
-------------------------
The CollectiveCompute carries a ~15us fixed cost, so the kernel is built
around issuing it as early as possible (~5.4us) and filling its window:

  A (0..5.4us)   x loads on SP+gpsimd (4 chunks) -> local stats: min via
                 DVE strided chunk-reduces, max via gpsimd two-stage
                 cross-lane C-max (with a DRAM bounce to put t on
                 partitions) -> (-min | max+eps) replicated [8,128] ->
                 ReduceScatter(max) folds the 8 cores in the collective.
  B (window)     bins softmax/cumsum, EW/D10 matmuls, fp8 D8/R8 residual
                 pair construction, PE transposes of x into u_t + ACT
                 evictions, PE p-state kept hot with spin transposes.
  C (post)       tiny gathers -> threshold chain (gpsimd tensor_scalar) ->
                 9 indicator passes (DVE is_gt: 3,4,5q,6q; gpsimd is_gt:
                 0q,1q,8q; ACT Sign form: 2,7) feeding 36 fp8 DoubleRow
                 matmuls (each pair contracts D8_k|R8_k against the same
                 indicator via a broadcast rhs), bf16 evictions + stores.

All cross-queue waits are scheduled to arrive after their producer DMAs
complete (blocked waits pay the full DMA pipeline latency in the cost
model); per-queue fillers keep the hot consumers from blocking early.
"""

import numpy as np

import concourse.bass as bass
import concourse.mybir as mybir
import concourse.tile as tile
from concourse import bacc, bass_utils

F32 = mybir.dt.float32
F32R = mybir.dt.float32r
F8 = mybir.dt.float8e4
BF16 = mybir.dt.bfloat16
ALU = mybir.AluOpType
AX = mybir.AxisListType
ACT = mybir.ActivationFunctionType
MMPM = mybir.MatmulPerfMode

B_FULL, IN_DIM, NUM_BINS, EMBED_DIM = 32768, 64, 10, 64
N_CORES = 8
B_C = B_FULL // N_CORES          # 4096 rows per core
EPS = 1e-6
P = 128
T_ALL = B_C // P                 # 32 row-groups (t index)
NTHR = NUM_BINS - 1              # 9 real thresholds
UCOLS = B_C // 2                 # 2048 u columns (2 rows per column)
MM_N = 512                       # matmul moving free size (one PSUM bank)
N_SPIN = 114                     # PE keep-warm transposes during collective
SGN_BINS = (2, 7)                # bins evaluated on ACT via Sign (+-1 form)
QUARTERED = (0, 1, 5, 6, 8)      # bins produced at quarter granularity
KORDER = (3, 0, 2, 4, 1, 5, 7, 6, 8)   # PE consumption order
ND10 = NUM_BINS                  # d10 rows: 9 diffs + base


def _mt_np():
    """MT [11, 10]: cols 0..8 bin diffs (sign bins halved), col 9 the base
    row 64*(EW0+b) + sign-form corrections 32*D_k."""
    mt = np.zeros((NUM_BINS + 1, NUM_BINS), dtype=np.float32)
    for k in range(NTHR):
        mt[k, k] = -1.0
        mt[k + 1, k] = 1.0
    mt[0, NTHR] = float(IN_DIM)
    mt[NUM_BINS, NTHR] = float(IN_DIM)
    for k in SGN_BINS:
        mt[k, k] = -0.5
        mt[k + 1, k] = 0.5
        mt[k, NTHR] += -float(IN_DIM) / 2
        mt[k + 1, NTHR] += float(IN_DIM) / 2
    return mt


def _trace_kernel(tc, io, tag=""):
    nc = tc.nc
    x_d, bl_d, emb_d, w_d, b_d, out_d, ident_d, mt_d, zero_d = io

    with (
        tc.tile_pool(name=f"const{tag}", bufs=1) as cpool,
        tc.tile_pool(name=f"ind{tag}", bufs=9) as ipool,
        tc.tile_pool(name=f"outs{tag}", bufs=4) as opool,
        tc.tile_pool(name=f"tp_psum{tag}", bufs=2, space="PSUM") as tp_psum,
        tc.tile_pool(name=f"out_psum{tag}", bufs=4, space="PSUM") as out_psum,
        tc.tile_pool(name=f"mc_psum{tag}", bufs=2, space="PSUM") as mc_psum,
        tc.tile_pool(name=f"dram{tag}", bufs=1, space="DRAM") as dpool,
    ):
        # ---------------- Phase A: x load + local stats + collective --------
        # x natural layout: partition p holds batch rows p*32+t, 8KB each.
        # ACT's queue head carries the one-time LoadActFuncSet (~1.3us), so
        # x rides only SP + gpsimd.  Chunk sizes are tuned so (a) SP's
        # chunks land early for the DVE max path, (b) gpsimd's own last
        # chunk ends AFTER SP's chunks have completed, so the min tree
        # reaching the gpsimd queue head never blocks on a DMA semaphore
        # (a blocked wait pays the full DMA pipeline latency).
        x_nat = cpool.tile([P, T_ALL * IN_DIM], F32)
        x_view = x_d.rearrange("(p t) n -> p (t n)", p=P)
        xb = (0, 448, 960, 1472, 2048)
        for c, q in ((0, nc.sync), (1, nc.sync), (2, nc.gpsimd), (3, nc.gpsimd)):
            sl = slice(xb[c], xb[c + 1])
            q.dma_start(x_nat[:, sl], x_view[:, sl])
        # gpsimd filler so its first stat op arrives at the queue head
        # after SP's chunks have finished (blocked waits pay full latency)
        pfill = cpool.tile([1, 64], F32)
        nc.gpsimd.memset(pfill[:], 0.0)
        for _ in range(2):
            nc.gpsimd.tensor_scalar(pfill[:], pfill[:], 1.0, None, ALU.mult)

        # small param DMAs during the x-DMA latency window
        bl_row = cpool.tile([1, NUM_BINS], F32)
        nc.scalar.dma_start(bl_row[:], bl_d.unsqueeze(0))
        ew_ext = cpool.tile([NUM_BINS + 1, EMBED_DIM], F32)
        nc.scalar.dma_start(ew_ext[NUM_BINS : NUM_BINS + 1, :], b_d.unsqueeze(0))
        emb_s = cpool.tile([NUM_BINS, EMBED_DIM], F32)
        nc.scalar.dma_start(emb_s[:], emb_d)
        w_s = cpool.tile([EMBED_DIM, EMBED_DIM], F32)
        nc.sync.dma_start(w_s[:], w_d)

        # ident / MT are constants embedded in the NEFF, DMA'd in early.
        # MT [11, 10]: D10 = MT.T @ ew_ext (bidiagonal diffs; col 9 =
        # 64*(row0+rowb)).  Bin KSGN is evaluated on ACT as sign(u-T) in
        # {-1,1}, i.e. ind = (sign+1)/2, so its D row is halved and
        # 32*D_ksgn is folded into the base row.
        ident = cpool.tile([P, P], F32)
        nc.sync.dma_start(ident[:], ident_d)
        mt = cpool.tile([NUM_BINS + 1, ND10], F32)
        nc.scalar.dma_start(mt[:], mt_d)

        # DVE filler: a blocked semaphore wait wakes only at the producer's
        # full DMA pipeline latency, while an instruction arriving at the
        # queue head after the DMA finished proceeds immediately — so keep
        # DVE busy until the first x chunk has landed.
        filler = cpool.tile([1, 720], F32)
        nc.vector.memset(filler[:], 0.0)

        # min path on DVE: strided chunk reduces over t, merge, negate
        # (HW cross-lane reduce has no min, DVE free-axis reduce does)
        stat_min = cpool.tile([P, 4 * IN_DIM], F32)
        for i, c in enumerate((0, 2, 1, 3)):
            nc.vector.tensor_reduce(
                stat_min[:, i * IN_DIM : (i + 1) * IN_DIM],
                x_nat[:, xb[c] : xb[c + 1]].rearrange(
                    "p (t n) -> p n t", n=IN_DIM
                ),
                AX.X, ALU.min,
            )
        negmin = cpool.tile([P, IN_DIM], F32)
        nc.vector.tensor_reduce(
            negmin[:],
            stat_min[:].rearrange("p (c n) -> p n c", n=IN_DIM),
            AX.X, ALU.min,
        )
        nc.vector.tensor_scalar(negmin[:], negmin[:], -1.0, None, ALU.mult)

        # max path on gpsimd: cross-lane C-max folds the 128 partitions,
        # an SBUF->SBUF scatter puts the 32 t-groups on partitions, and a
        # second C-max folds those.
        cmax_row = cpool.tile([1, T_ALL * IN_DIM], F32)
        nc.gpsimd.tensor_reduce(cmax_row[:], x_nat[:], AX.C, ALU.max)
        cmr_d = dpool.tile([1, T_ALL * IN_DIM], F32)
        nc.gpsimd.dma_start(cmr_d[:], cmax_row[:])
        cmax32 = cpool.tile([T_ALL, IN_DIM], F32)
        nc.gpsimd.dma_start(
            cmax32[:], cmr_d[:].rearrange("o (t n) -> (o t) n", n=IN_DIM)
        )
        # gmm = (-min | max+eps), replicated to [8,128] in DRAM so the
        # cross-core reduce is a single ReduceScatter(max)
        gmm = cpool.tile([1, P], F32)
        nc.gpsimd.tensor_reduce(gmm[:, IN_DIM:P], cmax32[:], AX.C, ALU.max)
        nc.gpsimd.tensor_scalar(
            gmm[:, IN_DIM:P], gmm[:, IN_DIM:P], 1.0, EPS, ALU.mult, ALU.add
        )
        nc.gpsimd.tensor_reduce(gmm[:, 0:IN_DIM], negmin[:], AX.C, ALU.max)

        # stage to DRAM (replicated 8x) and ReduceScatter(max), both on
        # the gpsimd queue: same-queue waits resolve at the producer's
        # processing end, so the collective doesn't pay the DMA latency.
        # The RS folds the 8 cores' stats, so cc_out is already global.
        cc_in = dpool.tile([N_CORES, P], F32)
        nc.gpsimd.dma_start(
            cc_in[:].rearrange("r n -> () r n"),
            gmm[:].unsqueeze(1).broadcast_to([1, N_CORES, P]),
        )
        cc_out = dpool.tile([1, P], F32)
        nc.gpsimd.collective_compute(
            "ReduceScatter",
            ALU.max,
            replica_groups=[list(range(N_CORES))],
            ins=[cc_in[:]],
            outs=[cc_out[:]],
        )
        # gather into per-partition layout: post2[(h,n), s] = cc_out[0, s*64+n]
        post2 = cpool.tile([P, 2], F32)
        src_ap = cc_out[:].rearrange("o (s n) -> (o n) s", s=2)
        for h in (0, 1):
            nc.gpsimd.dma_start(post2[h * IN_DIM : (h + 1) * IN_DIM, :], src_ap)

        # ---------------- Phase B: window work ------------------------------
        # bins = cumsum(softmax(bin_logits))  (DVE after its max-reduce)
        e_row = cpool.tile([1, NUM_BINS], F32)
        nc.scalar.activation(e_row[:], bl_row[:], ACT.Exp)
        ssum = cpool.tile([1, 1], F32)
        nc.vector.tensor_reduce(ssum[:], e_row[:], AX.X, ALU.add)
        rsum = cpool.tile([1, 1], F32)
        nc.vector.reciprocal(rsum[:], ssum[:])
        prob_a = cpool.tile([1, NUM_BINS], F32)
        nc.vector.tensor_scalar(prob_a[:], e_row[:], rsum[:, 0:1], None, ALU.mult)
        prob_b = cpool.tile([1, NUM_BINS], F32)
        cur, nxt = prob_a, prob_b
        for sh in (1, 2, 4, 8):
            nc.vector.tensor_copy(nxt[:, 0:sh], cur[:, 0:sh])
            nc.vector.tensor_tensor(
                nxt[:, sh:NUM_BINS], cur[:, sh:NUM_BINS], cur[:, 0 : NUM_BINS - sh],
                ALU.add,
            )
            cur, nxt = nxt, cur
        # bins broadcast to every partition via DRAM bounce (ACT queue)
        bins_d = dpool.tile([1, NUM_BINS], F32)
        nc.scalar.dma_start(bins_d[:], cur[:])
        bins_bc = cpool.tile([P, NUM_BINS], F32)
        nc.scalar.dma_start(bins_bc[:], bins_d[:].broadcast_to([P, NUM_BINS]))

        # EW = embed @ W.T ; D10 rows 0..8 = EW[k+1]-EW[k], row 9 = 64*(EW[0]+b)
        ps_embT = mc_psum.tile([EMBED_DIM, NUM_BINS], F32, tag="mc")
        nc.tensor.transpose(ps_embT[:], emb_s[:], ident[0:NUM_BINS, 0:NUM_BINS])
        embT_s = cpool.tile([EMBED_DIM, NUM_BINS], F32)
        nc.scalar.activation(embT_s[:], ps_embT[:], ACT.Copy)

        ps_wt = mc_psum.tile([EMBED_DIM, EMBED_DIM], F32, tag="mc")
        nc.tensor.transpose(ps_wt[:], w_s[:], ident[0:EMBED_DIM, 0:EMBED_DIM])
        wt_s = cpool.tile([EMBED_DIM, EMBED_DIM], F32)
        nc.scalar.activation(wt_s[:], ps_wt[:], ACT.Copy)

        ps_ew = mc_psum.tile([NUM_BINS, EMBED_DIM], F32, tag="mc")
        nc.tensor.matmul(ps_ew[:], embT_s[:], wt_s[:], start=True, stop=True)
        nc.scalar.activation(ew_ext[0:NUM_BINS, :], ps_ew[:], ACT.Copy)

        ps_d10 = mc_psum.tile([ND10, EMBED_DIM], F32, tag="mc")
        nc.tensor.matmul(ps_d10[:], mt[:], ew_ext[:], start=True, stop=True)
        d10_s = cpool.tile([ND10, EMBED_DIM], F32)
        nc.scalar.activation(d10_s[:], ps_d10[:], ACT.Copy)
        d10_d = dpool.tile([ND10, EMBED_DIM], F32)
        nc.scalar.dma_start(d10_d[:], d10_s[:])

        # fp8 weights with residual compensation: D8 = fp8(D), R8 = fp8(D-D8).
        # Each DoubleRow matmul contracts (D8_k | R8_k) against the same
        # indicator, recovering ~full precision at 0.5 cycles/row.
        d8_s = cpool.tile([ND10, EMBED_DIM], F8)
        nc.scalar.activation(d8_s[:], d10_s[:], ACT.Copy)
        d8f_s = cpool.tile([ND10, EMBED_DIM], F32)
        nc.vector.tensor_copy(d8f_s[:], d8_s[:])
        r_s = cpool.tile([ND10, EMBED_DIM], F32)
        nc.vector.tensor_tensor(
            r_s[:], d10_s[:], d8f_s[:], ALU.subtract
        )
        r8_s = cpool.tile([ND10, EMBED_DIM], F8)
        nc.vector.tensor_copy(r8_s[:], r_s[:])
        d8_d = dpool.tile([ND10, EMBED_DIM], F8)
        nc.scalar.dma_start(d8_d[:], d8_s[:])
        r8_d = dpool.tile([ND10, EMBED_DIM], F8)
        nc.scalar.dma_start(r8_d[:], r8_s[:])

        # block-diagonal fp8 D tiles: dblk8 [128=(par,n), (k, i, par', f)]
        # (i = 0 -> D8, i = 1 -> R8), nonzero only where par' == par, built
        # from a [2, 2304] master pair and one partition-broadcast DMA.
        # The zero skeleton comes from an inline constant (no memset — a
        # dep-free memset gets hoisted to the DVE queue head and delays
        # the stats chain); data rows are then DMA'd over it.
        dblk8 = cpool.tile([P, NTHR * 2 * P], F8)
        mini = cpool.tile([2, NTHR * 2 * P], F8)
        nc.sync.dma_start(mini[:], zero_d)
        for h in range(2):
            mrow = mini[h : h + 1, :].rearrange(
                "o (k i g f) -> o k i g f", k=NTHR, i=2, g=2
            )
            nc.sync.dma_start(mrow[:, :, 0, h, :],
                              d8_d[0:NTHR, :].unsqueeze(0))
            nc.sync.dma_start(mrow[:, :, 1, h, :],
                              r8_d[0:NTHR, :].unsqueeze(0))
        nc.sync.dma_start(
            dblk8[:],
            mini[:].unsqueeze(1).broadcast_to([2, 64, NTHR * 2 * P]),
        )
        # base bias per (par, f) partition: 64*(EW[0]+b) + sign-bin shifts
        base_col = cpool.tile([P, 1], F32)
        brow = d10_d[NTHR : NTHR + 1, :].squeeze(0).unsqueeze(1)  # [64, 1]
        for h in range(2):
            nc.scalar.dma_start(base_col[h * 64 : (h + 1) * 64, :], brow)

        # PE transposes x into u_t [128=(par,n), 2048=(j,p)]
        u_t = cpool.tile([P, UCOLS], F32)
        for g in range(4):
            ps_tp = tp_psum.tile([P, 4 * P], F32, tag="tp")
            for jj in range(4):
                j = g * 4 + jj
                nc.tensor.transpose(
                    ps_tp[:, jj * P : (jj + 1) * P],
                    x_nat[:, j * P : (j + 1) * P],
                    ident[:],
                )
            nc.scalar.activation(
                u_t[:, g * 4 * P : (g + 1) * 4 * P], ps_tp[:], ACT.Copy
            )

        # keep the PE p-state ramped through the collective window
        spin_ps = mc_psum.tile([P, P], F32, tag="mc")
        for _ in range(N_SPIN):
            nc.tensor.transpose(spin_ps[:], ident[:], ident[:])

        # ---------------- Phase C: thresholds + indicators + matmul ---------
        # post2 = (-gmin | gmax+eps) per partition; all gpsimd tensor_scalar
        range_dup = cpool.tile([P, 1], F32)
        nc.gpsimd.tensor_scalar(
            range_dup[:], post2[:, 1:2], 1.0, post2[:, 0:1], ALU.mult, ALU.add
        )
        # thresholds: s_thr[(par,n), k] = bins[k]*(range[n]+eps) - (-min[n])
        s_thr = cpool.tile([P, NUM_BINS], F32)
        nc.gpsimd.tensor_scalar(
            s_thr[:], bins_bc[:], range_dup[:, 0:1], post2[:, 0:1],
            ALU.mult, ALU.subtract,
        )
        thr_neg = cpool.tile([P, NUM_BINS], F32)
        nc.gpsimd.tensor_scalar(
            thr_neg[:], s_thr[:], -1.0, None, ALU.mult
        )

        # indicator producers: DVE k0(quartered),2,4,6; gpsimd k1(quartered),
        # 3,5; ACT bins 7,8 as sign(u - T) in {-1, 1}
        n_sub = UCOLS // MM_N  # 4
        ps_out = [out_psum.tile([P, MM_N], F32, tag="out", name=f"pso_{s}")
                  for s in range(n_sub)]
        # producer assignment (from the static schedule search):
        #   ACT (sign form): bins 2, 7 full passes
        #   DVE: bins 3, 4 full; 5, 6 quartered
        #   Pool: bins 0, 1, 8 quartered
        POOL_BINS = (0, 1, 8)
        inds = {}
        for k in range(NTHR):
            inds[k] = ipool.tile([P, UCOLS], F8, tag="ind", name=f"ind_{k}")
        for k in (0, 1, 8):
            for q in range(4):
                qsl = slice(q * MM_N, (q + 1) * MM_N)
                nc.gpsimd.tensor_scalar(
                    inds[k][:, qsl], u_t[:, qsl], s_thr[:, k : k + 1], None,
                    ALU.is_gt,
                )
        for k in (3, 4):
            nc.vector.tensor_scalar(
                inds[k][:], u_t[:], s_thr[:, k : k + 1], None, ALU.is_gt
            )
        for k in (5, 6):
            for q in range(4):
                qsl = slice(q * MM_N, (q + 1) * MM_N)
                nc.vector.tensor_scalar(
                    inds[k][:, qsl], u_t[:, qsl], s_thr[:, k : k + 1], None,
                    ALU.is_gt,
                )
        for k in SGN_BINS:
            nc.scalar.activation(
                inds[k][:], u_t[:], ACT.Sign, bias=thr_neg[:, k : k + 1]
            )
        # PE consumes pairs (D8_k | R8_k) x (ind_k, ind_k) via DoubleRow,
        # in expected-readiness order; the last two bins are sub-interleaved
        # so evictions/stores cascade early.
        korder = KORDER

        def pair_mm(k, sub, start, stop, slot=None):
            slot = k if slot is None else slot
            lhsT = dblk8[:, slot * 2 * P : (slot + 1) * 2 * P].rearrange(
                "p (i m) -> p i m", i=2
            )
            rhs = (
                inds[k][:, sub * MM_N : (sub + 1) * MM_N]
                .unsqueeze(1)
                .broadcast_to([P, 2, MM_N])
            )
            nc.tensor.matmul(
                ps_out[sub][:], lhsT, rhs,
                start=start, stop=stop, perf_mode=MMPM.DoubleRow,
            )

        for ki, k in enumerate(korder[:7]):
            for sub in range(n_sub):
                pair_mm(k, sub, start=(ki == 0), stop=False)
        # interleaved tail: each sub's accumulation closes as early as its
        # last quarters land, cascading evictions/stores
        for sub in range(n_sub):
            pair_mm(korder[7], sub, start=False, stop=False)
            pair_mm(korder[8], sub, start=False, stop=True)

        # biased bf16 evictions + stores (tail matmuls run sub order
        # 3,2,1,0; bf16 halves the store bytes and the rounding is well
        # inside the error budget):
        #   sub3: DVE evict -> SP dma   sub2: ACT evict -> ACT dma
        #   sub1: Pool evict -> Pool dma  sub0: Pool evict -> SP dma
        out_s = [opool.tile([P, MM_N], BF16, tag="outs", name=f"outs_{s}")
                 for s in range(n_sub)]
        nc.scalar.activation(
            out_s[0][:], ps_out[0][:], ACT.Identity, bias=base_col[:, 0:1]
        )
        nc.sync.dma_start(out_d[:, 0:MM_N], out_s[0][:])
        nc.vector.tensor_scalar(
            out_s[1][:], ps_out[1][:], base_col[:, 0:1], None, ALU.add
        )
        nc.gpsimd.dma_start(out_d[:, MM_N : 2 * MM_N], out_s[1][:])
        nc.scalar.activation(
            out_s[2][:], ps_out[2][:], ACT.Identity, bias=base_col[:, 0:1]
        )
        nc.gpsimd.dma_start(out_d[:, 2 * MM_N : 3 * MM_N], out_s[2][:])
        nc.vector.tensor_scalar(
            out_s[3][:], ps_out[3][:], base_col[:, 0:1], None, ALU.add
        )
        nc.sync.dma_start(out_d[:, 3 * MM_N : 4 * MM_N], out_s[3][:])


_CACHED = {}


def _build(loop=1):
    if loop in _CACHED:
        return _CACHED[loop]
    nc = bacc.Bacc(
        "TRN2",
        target_bir_lowering=False,
        debug=False,
        enable_asserts=True,
        num_devices=N_CORES,
    )
    with tile.TileContext(nc) as tc:
        io = (
            nc.dram_tensor("x_sh", [B_C, IN_DIM], F32, kind="ExternalInput").ap(),
            nc.dram_tensor("bin_logits", [NUM_BINS], F32, kind="ExternalInput").ap(),
            nc.dram_tensor("embed", [NUM_BINS, EMBED_DIM], F32, kind="ExternalInput").ap(),
            nc.dram_tensor("W", [EMBED_DIM, EMBED_DIM], F32, kind="ExternalInput").ap(),
            nc.dram_tensor("b", [EMBED_DIM], F32, kind="ExternalInput").ap(),
            nc.dram_tensor("out_t", [P, UCOLS], BF16, kind="ExternalOutput").ap(),
            nc.inline_tensor(np.eye(P, dtype=np.float32), "ident_c").ap(),
            nc.inline_tensor(_mt_np(), "mt_c").ap(),
            nc.inline_tensor(
                np.zeros((2, NTHR * 2 * P), dtype=np.float32), "zero_c"
            ).ap().bitcast(F8)[:, 0 : NTHR * 2 * P],
        )
        for it in range(loop):
            _trace_kernel(tc, io, tag=f"_{it}" if loop > 1 else "")
    nc.compile()
    _CACHED[loop] = nc
    return nc


def _make_in_maps(x, bin_logits, embed, W, b):
    maps = []
    for c in range(N_CORES):
        maps.append(
            {
                "x_sh": np.ascontiguousarray(x[c * B_C : (c + 1) * B_C]),
                "bin_logits": np.asarray(bin_logits),
                "embed": np.asarray(embed),
                "W": np.asarray(W),
                "b": np.asarray(b),
            }
        )
    return maps


def _unshard(results):
    shards = []
    for c in range(N_CORES):
        out_t = np.asarray(results[c]["out_t"], dtype=np.float32)
        shard = (
            out_t.reshape(2, EMBED_DIM, T_ALL // 2, P)
            .transpose(3, 2, 0, 1)           # [p, j, par, f]
            .reshape(B_C, EMBED_DIM)         # b = p*32 + j*2 + par
        )
        shards.append(shard)
    return np.ascontiguousarray(np.concatenate(shards, axis=0))


def kernel(x, bin_logits, embed, W, b):
    nc = _build()
    in_maps = _make_in_maps(np.asarray(x, dtype=np.float32), bin_logits, embed, W, b)
    res = bass_utils.run_bass_kernel_spmd(nc, in_maps, core_ids=list(range(N_CORES)))
    return _unshard(res.results)

